# revision 1
# baseline (speedup 1.0000x reference)
"""Causal depthwise conv1d (B=8, S=4096, H=2048, KS=4) on 8 trn2 NeuronCores.

Strategy:
  - Shard batch across the 8 cores (one batch element each, no halo needed).
  - Host-side transpose each batch element to (H, S) so the device sees
    channels on SBUF partitions and the sequence dim contiguous on the free
    axis: fully coalesced f32 DMA both ways, conv shifts become free-dim AP
    offsets, and the per-channel weights become per-partition [P,1] scalars.
  - Per 128-channel block: two ACT passes (w3*x + bias, w2*x_{-1}) and three
    DVE passes (two fused scalar_tensor_tensor multiply-adds + one add).
"""

import numpy as np

B, S, H, KS = 8, 4096, 2048, 4
NCORES = 8
PB = 128            # SBUF partitions
HB = H // PB        # 16 channel blocks per core
PAD = 4             # left zero-pad columns in the x tile (3 used + 1 align)

# test.py can inject e.g. {"trace": True, "tmpdir": ...} here.
RUN_KWARGS = {}
LAST_RESULTS = []   # last BassKernelResults, for the harness to inspect

_cached = {}


def _build():
    import concourse.bacc as bacc
    import concourse.mybir as mybir
    import concourse.tile as tile

    f32 = mybir.dt.float32
    Alu = mybir.AluOpType
    Act = mybir.ActivationFunctionType

    nc = bacc.Bacc(
        "TRN2",
        target_bir_lowering=False,
        debug=False,
        num_devices=NCORES,
    )
    xT = nc.dram_tensor("xT", [H, S], f32, kind="ExternalInput")
    wp = nc.dram_tensor("wp", [PB, HB * 5], f32, kind="ExternalInput")
    yT = nc.dram_tensor("yT", [H, S], f32, kind="ExternalOutput")

    SC = S // 2  # load-split width
    with tile.TileContext(nc) as tc:
        with tc.tile_pool(name="wpool", bufs=1) as wpool, \
             tc.tile_pool(name="xpool", bufs=3) as xpool, \
             tc.tile_pool(name="data", bufs=4) as pool:
            wsb = wpool.tile([PB, HB * 5], f32)
            # scalar ring: keeps the sync-ring FIFO clear for the first x load
            nc.scalar.dma_start(wsb[:], wp[:])
            # Tiny no-dep ACTIVATE so the ACT table load overlaps the first
            # x DMA instead of serializing in front of the first product.
            warm = wpool.tile([PB, 2], f32)
            nc.vector.memset(warm[:], 0.0)
            nc.scalar.activation(warm[:], warm[:], Act.Identity, bias=0.0,
                                 scale=1.0)
            for hb in range(HB):
                rows = slice(hb * PB, (hb + 1) * PB)
                xt = xpool.tile([PB, PAD + S], f32)
                nc.vector.memset(xt[:, 0:PAD], 0.0)
                c = hb * 5
                w0 = wsb[:, c + 0:c + 1]
                w1 = wsb[:, c + 1:c + 2]
                w2 = wsb[:, c + 2:c + 3]
                w3 = wsb[:, c + 3:c + 4]
                bb = wsb[:, c + 4:c + 5]

                # First tile: fine chunks so compute starts ~3us after t0.
                # Last tile: split so the final store is half-size (shorter
                # tail). Middle tiles: full-width ops (min DVE overhead).
                if hb == 0:
                    chunks = [S // 8] * 2 + [S // 4] + [S // 2]
                elif hb == HB - 1:
                    chunks = [S // 2, S // 4, S // 4]
                else:
                    chunks = [S]
                s0 = 0
                for ci, cw in enumerate(chunks):
                    base = PAD + s0
                    nc.sync.dma_start(xt[:, base:base + cw],
                                      xT[rows, s0:s0 + cw])
                    t3 = pool.tile([PB, S], f32, tag="t3", bufs=5)
                    t2 = pool.tile([PB, S], f32, tag="t2", bufs=3)
                    if hb == 0 and ci == 0:
                        # products on DVE (2x-mode tensor_scalar) so the very
                        # first compute has no ACT table-load/product in its
                        # critical path
                        nc.vector.tensor_scalar(t3[:, :cw], xt[:, base:base + cw],
                                                w3, bb, op0=Alu.mult,
                                                op1=Alu.add)
                        nc.vector.tensor_scalar(t2[:, :cw],
                                                xt[:, base - 1:base - 1 + cw],
                                                w2, None, op0=Alu.mult)
                    else:
                        # t3 = w3 * x[s] + bias
                        nc.scalar.activation(t3[:, :cw], xt[:, base:base + cw],
                                             Act.Identity, bias=bb, scale=w3)
                        # t2 = w2 * x[s-1]
                        nc.scalar.activation(t2[:, :cw],
                                             xt[:, base - 1:base - 1 + cw],
                                             Act.Copy, scale=w2)
                    # t3 += w0 * x[s-3]
                    nc.vector.scalar_tensor_tensor(
                        t3[:, :cw], xt[:, base - 3:base - 3 + cw], w0,
                        t3[:, :cw], op0=Alu.mult, op1=Alu.add)
                    # t2 += w1 * x[s-2]
                    nc.vector.scalar_tensor_tensor(
                        t2[:, :cw], xt[:, base - 2:base - 2 + cw], w1,
                        t2[:, :cw], op0=Alu.mult, op1=Alu.add)
                    # t3 = t3 + t2, stored from the same tile
                    nc.vector.tensor_tensor(t3[:, :cw], t3[:, :cw], t2[:, :cw],
                                            op=Alu.add)
                    nc.scalar.dma_start(yT[rows, s0:s0 + cw], t3[:, :cw])
                    s0 += cw
    nc.compile()
    return nc


def get_nc():
    if "nc" not in _cached:
        _cached["nc"] = _build()
    return _cached["nc"]


def pack_weights(weight, bias):
    wp = np.empty((PB, HB * 5), dtype=np.float32)
    for hb in range(HB):
        sl = slice(hb * PB, (hb + 1) * PB)
        for k in range(KS):
            wp[:, hb * 5 + k] = weight[k, sl]
        wp[:, hb * 5 + 4] = bias[sl]
    return wp


def kernel(x, weight, bias):
    from concourse.bass_utils import run_bass_kernel_spmd

    x = np.ascontiguousarray(np.asarray(x, dtype=np.float32))
    weight = np.asarray(weight, dtype=np.float32)
    bias = np.asarray(bias, dtype=np.float32)
    assert x.shape == (B, S, H), x.shape
    assert weight.shape == (KS, H), weight.shape
    assert bias.shape == (H,), bias.shape

    nc = get_nc()
    wp = pack_weights(weight, bias)
    xT = np.ascontiguousarray(x.transpose(0, 2, 1))  # (B, H, S)

    in_maps = [{"xT": xT[i], "wp": wp} for i in range(NCORES)]
    try:
        res = run_bass_kernel_spmd(nc, in_maps, core_ids=list(range(NCORES)),
                                   **RUN_KWARGS)
    except Exception:
        # one retry for transient device hiccups
        res = run_bass_kernel_spmd(nc, in_maps, core_ids=list(range(NCORES)),
                                   **RUN_KWARGS)
    LAST_RESULTS.clear()
    LAST_RESULTS.append(res)
    y = np.stack([res.results[i]["yT"] for i in range(NCORES)])  # (B, H, S)
    return np.ascontiguousarray(y.transpose(0, 2, 1))



# revision 6
# speedup vs baseline: 1.6743x; 1.6743x over previous
"""Causal depthwise conv1d (B=8, S=4096, H=2048, KS=4) on 8 trn2 NeuronCores.

Strategy:
  - Shard batch across the 8 cores (one batch element each, no halo needed).
  - bf16 on the wire: host casts x to bf16 (and the result back to f32), so
    each core moves 16 MiB in + 16 MiB out instead of 32+32 — the kernel is
    DMA-bound, so halving bytes halves the roofline. bf16 rounding keeps the
    end-to-end rel err ~5e-3, well inside the 2e-2 gate.
  - Host-side transpose each batch element to (H, S): channels on SBUF
    partitions, sequence contiguous on the free axis. Conv shifts become
    free-dim AP offsets.
  - Engine split, measured per 2048-col half-block against the 2.9us DMA
    budget (DVE STT is 1x ~= 2.2us/2048, too slow to chain 3 taps on DVE):
      PE  : taps w0,w1,w2 as per-channel diagonal matmuls (bf16, 1 col/cyc)
            accumulating into a 4-bank PSUM tile          ~2.7us
      ACT : t = w3*x + bias (per-partition scale/bias)    ~2.1us
      DVE : t += psum  (tensor_tensor, PSUM operand)      ~2.4us
    PSUM: two 4-bank tiles ping-pong across half-blocks.
  - Stores are emitted two chunks late on the ACT ring so the ring never
    stalls waiting for a chunk's merge to finish.
"""

import numpy as np

B, S, H, KS = 8, 4096, 2048, 4
NCORES = 8
PB = 128            # SBUF partitions
HB = H // PB        # 16 channel blocks per core
PAD = 4             # left zero-pad columns in the x tile (3 used + 1 align)
CW = 2048           # half-block chunk width (4 PSUM banks of f32)
BANK = 512          # PSUM bank width in f32 elements
NPE = 3             # taps computed on PE (w0, w1, w2); w3 + bias on ACT

# test.py can inject e.g. {"trace": True, "tmpdir": ...} here.
RUN_KWARGS = {}
LAST_RESULTS = []   # last BassKernelResults, for the harness to inspect

_cached = {}


def _build():
    import concourse.bacc as bacc
    import concourse.mybir as mybir
    import concourse.tile as tile

    f32 = mybir.dt.float32
    bf16 = mybir.dt.bfloat16
    Alu = mybir.AluOpType
    Act = mybir.ActivationFunctionType

    nc = bacc.Bacc(
        "TRN2",
        target_bir_lowering=False,
        debug=False,
        num_devices=NCORES,
    )
    xT = nc.dram_tensor("xT", [H, S], bf16, kind="ExternalInput")
    wp = nc.dram_tensor("wp", [PB, HB * 5], f32, kind="ExternalInput")
    wd = nc.dram_tensor("wd", [PB, HB * NPE * PB], bf16, kind="ExternalInput")
    yT = nc.dram_tensor("yT", [H, S], bf16, kind="ExternalOutput")

    with tile.TileContext(nc) as tc:
        with tc.tile_pool(name="wpool", bufs=1) as wpool, \
             tc.tile_pool(name="xpool", bufs=3) as xpool, \
             tc.tile_pool(name="data", bufs=4) as pool, \
             tc.tile_pool(name="ppool", bufs=2, space="PSUM") as ppool:
            wsb = wpool.tile([PB, HB * 5], f32)
            wdb = wpool.tile([PB, HB * NPE * PB], bf16)
            # scalar ring: keeps the sync-ring FIFO clear for the first x load
            nc.scalar.dma_start(wsb[:], wp[:])
            nc.scalar.dma_start(wdb[:], wd[:])
            # Tiny no-dep ACTIVATE so the ACT table load overlaps the first
            # x DMA instead of serializing in front of the first pass.
            warm = wpool.tile([PB, 2], bf16)
            nc.vector.memset(warm[:], 0.0)
            nc.scalar.activation(warm[:], warm[:], Act.Identity, bias=0.0,
                                 scale=1.0)

            # chunk list: (hb, s0) half-blocks in order
            chunks = []
            for hb in range(HB):
                for s0 in range(0, S, CW):
                    chunks.append((hb, s0))
            NCH = len(chunks)

            xts = {}     # hb -> x tile
            state = {}   # chunk idx -> (t tile, hb, s0)

            for i in range(NCH + 2):
                if i < NCH:
                    hb, s0 = chunks[i]
                    rows = slice(hb * PB, (hb + 1) * PB)
                    if s0 == 0:
                        xt = xpool.tile([PB, PAD + S], bf16)
                        xts[hb] = xt
                        nc.vector.memset(xt[:, 0:PAD], 0.0)
                    xt = xts[hb]
                    base = PAD + s0
                    nc.sync.dma_start(xt[:, base:base + CW],
                                      xT[rows, s0:s0 + CW])
                    c = hb * 5
                    w3 = wsb[:, c + 3:c + 4]
                    bb = wsb[:, c + 4:c + 5]
                    ps = ppool.tile([PB, CW], f32)
                    # PE: psum = w0*x[s-3] + w1*x[s-2] + w2*x[s-1]
                    # tap-major so the stationary diag is loaded 3x per chunk
                    for k in range(NPE):
                        dcol = (hb * NPE + k) * PB
                        dw = wdb[:, dcol:dcol + PB]
                        shift = base - (NPE - k)  # k=0 -> s-3 ... k=2 -> s-1
                        for b in range(CW // BANK):
                            nc.tensor.matmul(
                                ps[:, b * BANK:(b + 1) * BANK],
                                dw,
                                xt[:, shift + b * BANK:shift + (b + 1) * BANK],
                                start=(k == 0), stop=(k == NPE - 1),
                                skip_group_check=True)
                    # ACT: t = w3 * x[s] + bias
                    t = pool.tile([PB, CW], bf16, tag="t", bufs=4)
                    state[i] = (t, hb, s0)
                    nc.scalar.activation(t[:], xt[:, base:base + CW],
                                         Act.Identity, bias=bb, scale=w3)
                    # DVE: t += psum
                    nc.vector.tensor_tensor(t[:], t[:], ps[:], op=Alu.add)
                if i >= 2 and i - 2 < NCH:
                    t, hb, s0 = state.pop(i - 2)
                    rows = slice(hb * PB, (hb + 1) * PB)
                    nc.scalar.dma_start(yT[rows, s0:s0 + CW], t[:])
    nc.compile()
    return nc


def get_nc():
    if "nc" not in _cached:
        _cached["nc"] = _build()
    return _cached["nc"]


def pack_weights(weight, bias):
    wp = np.empty((PB, HB * 5), dtype=np.float32)
    for hb in range(HB):
        sl = slice(hb * PB, (hb + 1) * PB)
        for k in range(KS):
            wp[:, hb * 5 + k] = weight[k, sl]
        wp[:, hb * 5 + 4] = bias[sl]
    return wp


def pack_diag(weight):
    """Per-block diagonal matrices for taps w0..w2, bf16, [PB, HB*NPE*PB]."""
    import ml_dtypes
    wd = np.zeros((PB, HB * NPE * PB), dtype=ml_dtypes.bfloat16)
    idx = np.arange(PB)
    for hb in range(HB):
        for k in range(NPE):
            col = (hb * NPE + k) * PB
            wd[idx, col + idx] = weight[k, hb * PB + idx].astype(
                ml_dtypes.bfloat16)
    return wd


def kernel(x, weight, bias):
    import ml_dtypes
    from concourse.bass_utils import run_bass_kernel_spmd

    x = np.asarray(x, dtype=np.float32)
    weight = np.asarray(weight, dtype=np.float32)
    bias = np.asarray(bias, dtype=np.float32)
    assert x.shape == (B, S, H), x.shape
    assert weight.shape == (KS, H), weight.shape
    assert bias.shape == (H,), bias.shape

    nc = get_nc()
    wp = pack_weights(weight, bias)
    wd = pack_diag(weight)
    # (B, H, S) contiguous bf16 (astype of the transposed view emits C-order)
    xT = x.transpose(0, 2, 1).astype(ml_dtypes.bfloat16)

    in_maps = [{"xT": xT[i], "wp": wp, "wd": wd} for i in range(NCORES)]
    try:
        res = run_bass_kernel_spmd(nc, in_maps, core_ids=list(range(NCORES)),
                                   **RUN_KWARGS)
    except Exception:
        # one retry for transient device hiccups
        res = run_bass_kernel_spmd(nc, in_maps, core_ids=list(range(NCORES)),
                                   **RUN_KWARGS)
    LAST_RESULTS.clear()
    LAST_RESULTS.append(res)
    y = np.stack([res.results[i]["yT"] for i in range(NCORES)])  # (B, H, S)
    return y.transpose(0, 2, 1).astype(np.float32)


# revision 7
# speedup vs baseline: 1.8708x; 1.1173x over previous
"""Causal depthwise conv1d (B=8, S=4096, H=2048, KS=4) on 8 trn2 NeuronCores.

Strategy:
  - Shard batch across the 8 cores (one batch element each, no halo needed).
  - bf16 on the wire: host casts x to bf16 (and the result back to f32), so
    each core moves 16 MiB in + 16 MiB out instead of 32+32 — the kernel is
    DMA-bound, so halving bytes halves the roofline. bf16 rounding keeps the
    end-to-end rel err ~5e-3, well inside the 2e-2 gate.
  - Host-side transpose each batch element to (H, S): channels on SBUF
    partitions, sequence contiguous on the free axis. Conv shifts become
    free-dim AP offsets.
  - Engine split, per 2048-col half-block against the ~2.9us DMA budget:
      PE  : taps w0,w1,w2 as per-channel diagonal matmuls (bf16, ~218ns per
            512-col matmul with overlapped LDWEIGHTS)          ~2.6us
      ACT : t = w3*x + bias (per-partition scale/bias)         ~2.0us
      DVE : t += psum  (tensor_tensor, PSUM operand, 1x)       ~2.4us
  - PSUM in 1024-col (2-bank) tiles, bufs=4: a quarter's matmuls only wait
    on the TT four quarters back, keeping ~6us of WAR slack.
  - Ring hygiene (the previous revision lost 1.9us/chunk to DMA-semaphore
    reuse waits serializing the sync ring): loads are full-block 4096-col
    DMAs on the sync ring (half the issue traffic, 8KB descriptors); stores
    and the PAD memsets live on the otherwise-idle gpsimd SWDGE queue; the
    scalar ring only runs the ACT products.
"""

import numpy as np

B, S, H, KS = 8, 4096, 2048, 4
NCORES = 8
PB = 128            # SBUF partitions
HB = H // PB        # 16 channel blocks per core
PAD = 4             # left zero-pad columns in the x tile (3 used + 1 align)
HW_ = 2048          # half-block width (ACT / store granularity)
QW = 1024           # quarter width (PSUM tile = 2 banks)
BANK = 512          # PSUM bank width in f32 elements
NPE = 3             # taps computed on PE (w0, w1, w2); w3 + bias on ACT

RUN_KWARGS = {}
LAST_RESULTS = []

_cached = {}


def _build():
    import concourse.bacc as bacc
    import concourse.mybir as mybir
    import concourse.tile as tile

    f32 = mybir.dt.float32
    bf16 = mybir.dt.bfloat16
    Alu = mybir.AluOpType
    Act = mybir.ActivationFunctionType

    nc = bacc.Bacc(
        "TRN2",
        target_bir_lowering=False,
        debug=False,
        num_devices=NCORES,
    )
    xT = nc.dram_tensor("xT", [H, S], bf16, kind="ExternalInput")
    wp = nc.dram_tensor("wp", [PB, HB * 5], f32, kind="ExternalInput")
    wd = nc.dram_tensor("wd", [PB, HB * NPE * PB], bf16, kind="ExternalInput")
    yT = nc.dram_tensor("yT", [H, S], bf16, kind="ExternalOutput")

    with tile.TileContext(nc) as tc:
        with tc.tile_pool(name="wpool", bufs=1) as wpool, \
             tc.tile_pool(name="xpool", bufs=4) as xpool, \
             tc.tile_pool(name="data", bufs=6) as pool, \
             tc.tile_pool(name="ppool", bufs=4, space="PSUM") as ppool:
            wsb = wpool.tile([PB, HB * 5], f32)
            wdb = wpool.tile([PB, HB * NPE * PB], bf16)
            nc.scalar.dma_start(wsb[:], wp[:])
            nc.scalar.dma_start(wdb[:], wd[:])
            # Tiny no-dep ACTIVATE so the ACT table load overlaps the first
            # x DMA instead of serializing in front of the first product.
            warm = wpool.tile([PB, 2], bf16)
            nc.vector.memset(warm[:], 0.0)
            nc.scalar.activation(warm[:], warm[:], Act.Identity, bias=0.0,
                                 scale=1.0)

            halves = []   # (hb, s0) store/ACT units
            for hb in range(HB):
                for s0 in range(0, S, HW_):
                    halves.append((hb, s0))
            NH = len(halves)

            xts = {}
            state = {}    # half idx -> (t tile, hb, s0)

            for i in range(NH + 2):
                if i < NH:
                    hb, s0 = halves[i]
                    rows = slice(hb * PB, (hb + 1) * PB)
                    if s0 == 0:
                        # whole-block load; PAD memset on the idle gpsimd ring
                        xt = xpool.tile([PB, PAD + S], bf16)
                        xts[hb] = xt
                        nc.gpsimd.memset(xt[:, 0:PAD], 0.0)
                        nc.sync.dma_start(xt[:, PAD:PAD + S], xT[rows, :])
                    xt = xts[hb]
                    base = PAD + s0
                    c = hb * 5
                    w3 = wsb[:, c + 3:c + 4]
                    bb = wsb[:, c + 4:c + 5]
                    # ACT product for the whole half-block, issued before the
                    # matmuls so it starts as soon as the load lands
                    t = pool.tile([PB, HW_], bf16, tag="t", bufs=6)
                    state[i] = (t, hb, s0)
                    nc.scalar.activation(t[:], xt[:, base:base + HW_],
                                         Act.Identity, bias=bb, scale=w3)
                    for q in range(HW_ // QW):
                        qbase = base + q * QW
                        ps = ppool.tile([PB, QW], f32)
                        for k in range(NPE):
                            dcol = (hb * NPE + k) * PB
                            dw = wdb[:, dcol:dcol + PB]
                            shift = qbase - (NPE - k)  # k=0 -> s-3 .. k=2 -> s-1
                            for b in range(QW // BANK):
                                nc.tensor.matmul(
                                    ps[:, b * BANK:(b + 1) * BANK],
                                    dw,
                                    xt[:, shift + b * BANK:
                                           shift + (b + 1) * BANK],
                                    start=(k == 0), stop=(k == NPE - 1),
                                    skip_group_check=True)
                        # DVE: t[quarter] += psum
                        nc.vector.tensor_tensor(
                            t[:, q * QW:(q + 1) * QW],
                            t[:, q * QW:(q + 1) * QW], ps[:], op=Alu.add)
                if i >= 2 and i - 2 < NH:
                    t, hb, s0 = state.pop(i - 2)
                    rows = slice(hb * PB, (hb + 1) * PB)
                    nc.gpsimd.dma_start(yT[rows, s0:s0 + HW_], t[:])
    nc.compile()
    return nc


def get_nc():
    if "nc" not in _cached:
        _cached["nc"] = _build()
    return _cached["nc"]


def pack_weights(weight, bias):
    wp = np.empty((PB, HB * 5), dtype=np.float32)
    for hb in range(HB):
        sl = slice(hb * PB, (hb + 1) * PB)
        for k in range(KS):
            wp[:, hb * 5 + k] = weight[k, sl]
        wp[:, hb * 5 + 4] = bias[sl]
    return wp


def pack_diag(weight):
    """Per-block diagonal matrices for taps w0..w2, bf16, [PB, HB*NPE*PB]."""
    import ml_dtypes
    wd = np.zeros((PB, HB * NPE * PB), dtype=ml_dtypes.bfloat16)
    idx = np.arange(PB)
    for hb in range(HB):
        for k in range(NPE):
            col = (hb * NPE + k) * PB
            wd[idx, col + idx] = weight[k, hb * PB + idx].astype(
                ml_dtypes.bfloat16)
    return wd


def kernel(x, weight, bias):
    import ml_dtypes
    from concourse.bass_utils import run_bass_kernel_spmd

    x = np.asarray(x, dtype=np.float32)
    weight = np.asarray(weight, dtype=np.float32)
    bias = np.asarray(bias, dtype=np.float32)
    assert x.shape == (B, S, H), x.shape
    assert weight.shape == (KS, H), weight.shape
    assert bias.shape == (H,), bias.shape

    nc = get_nc()
    wp = pack_weights(weight, bias)
    wd = pack_diag(weight)
    xT = x.transpose(0, 2, 1).astype(ml_dtypes.bfloat16)

    in_maps = [{"xT": xT[i], "wp": wp, "wd": wd} for i in range(NCORES)]
    try:
        res = run_bass_kernel_spmd(nc, in_maps, core_ids=list(range(NCORES)),
                                   **RUN_KWARGS)
    except Exception:
        res = run_bass_kernel_spmd(nc, in_maps, core_ids=list(range(NCORES)),
                                   **RUN_KWARGS)
    LAST_RESULTS.clear()
    LAST_RESULTS.append(res)
    y = np.stack([res.results[i]["yT"] for i in range(NCORES)])
    return y.transpose(0, 2, 1).astype(np.float32)
